# revision 38
# baseline (speedup 1.0000x reference)
"""GAT (2-layer, 4-head + linear) Bass kernel for 8 Trainium2 NeuronCores.

Strategy (dst-sharded edge parallelism):
  - Nodes are sharded 8 ways. Each core owns all edges whose DESTINATION is
    in its shard, so segment softmax stats and message aggregation need no
    cross-core reduction.
  - Dense projections are data-parallel over node shards; per-node tables
    [features | softmax-logit terms] are AllGathered so every core can
    gather rows for arbitrary source nodes with indirect DMA.
  - Edge aggregation: edges are sorted by dst and grouped into chunks of 128
    destination nodes. Per 128-edge subtile a one-hot selection matrix S0
    (S0[e,d] = [dst_local(e)==d]) is built on the DVE; the TensorEngine then
    computes PSUM[d,:] += S0.T @ (gathered_rows * exp(leaky_relu(logit))),
    accumulating messages AND softmax denominators (via interleaved
    all-ones columns in the node table) in one matmul per subtile.
  - The reference's appended self loops are computed analytically per chunk
    from the core's own table rows (diagonal matmul), skipping the gather.
  - Latency hiding: each AllGather is split into two position-range halves;
    each half's gathers/partial aggregation overlap the other half's
    collective (part-1 partials park in SBUF). The per-edge destination
    logit tables (adp) are precomputed during the collectives from
    DVE-built transposed one-hots. Manual tile-scheduler phase waits keep
    the in-order engine queues from damming behind gather-dependent ops.
  - Tables, gathers and matmuls run in bf16 (PSUM accumulation, softmax
    logits and the output stay fp32); tolerance is 2e-2, measured ~6e-3.
  - Host-side rebalancing: each core's node list order (== chunk
    membership) is permuted so per-chunk edge counts are even across both
    table parts, minimizing the padded subtile count S (two passes since
    part membership depends on the assignment).

Self-contained: only needs numpy/ml_dtypes + the concourse (Bass) stack.
"""
import math
import numpy as np

# ---- problem dims (hardcoded per spec nn_GAT_27212912788335) ----
N_NODES = 50000
IN_CH = 128
HID = 64
HEADS = 4
OUT_CH = 64
NEG_SLOPE = 0.2
EPS = 1e-16
NCORES = 8
P = 128

C1 = HEADS * HID          # 256
# bf16 table rows; dma_gather requires elem bytes % 256 == 0
ROW1 = 384   # 4*(64 msg + 1 one) + asrc(4) + adst(4) + pad -> 768B rows
ROW2 = 128   # 64 msg + 1 one + asrc2 + pad -> 256B rows


def _cfg_full():
    NSH = N_NODES // NCORES               # 6250
    NCHUNK = math.ceil(NSH / P)           # 49
    NSHP = NCHUNK * P                     # 6272
    return dict(NC=NCORES, NSH=NSH, NSHP=NSHP, NCHUNK=NCHUNK)


# ---------------------------------------------------------------- host prep
def host_prep(inputs, cfg):
    """Build per-core input maps. Only layout/index work on the host — all
    floating-point math happens on device."""
    NC, NSH, NSHP, NCHUNK = cfg["NC"], cfg["NSH"], cfg["NSHP"], cfg["NCHUNK"]
    N = NC * NSH
    HALF = NC * NSHP // 2

    x = np.ascontiguousarray(np.asarray(inputs["x"], dtype=np.float32))
    ei = np.asarray(inputs["edge_index"], dtype=np.int64)
    W1 = np.asarray(inputs["W1"], dtype=np.float32)
    a_src1 = np.asarray(inputs["a_src1"], dtype=np.float32)
    a_dst1 = np.asarray(inputs["a_dst1"], dtype=np.float32)
    b1 = np.asarray(inputs["b1"], dtype=np.float32)
    W2 = np.asarray(inputs["W2"], dtype=np.float32)
    a_src2 = np.asarray(inputs["a_src2"], dtype=np.float32)
    a_dst2 = np.asarray(inputs["a_dst2"], dtype=np.float32)
    b2 = np.asarray(inputs["b2"], dtype=np.float32)
    Wl = np.asarray(inputs["Wl"], dtype=np.float32)
    bl = np.asarray(inputs["bl"], dtype=np.float32)

    # the reference appends one self loop per node; those are handled
    # analytically on-device (diagonal contribution from local rows), so
    # only the real edges go through the gather path
    src = ei[0].copy()
    dst = ei[1].copy()

    # ---- rebalance: reorder each core's node list so per-chunk edge counts
    # (split by src table PART: positions < PSPLIT go in the first AllGather
    # half, the rest in the second) are even. Chunk membership is just list
    # order, so this is free: xT columns, table rows, gather indices and the
    # final Y unshard all use the same host-chosen permutation. Part
    # membership depends on the assignment itself, so run two passes: pass 1
    # balances on a position estimate, pass 2 on pass-1 memberships.
    PSPLIT = (NCHUNK // 2 + 1) * P                # 3200: chunks 0..24 = part 1
    NP1 = PSPLIT
    NP2 = NSHP - PSPLIT
    deg = np.bincount(dst, minlength=N)
    newpos = np.empty(N, np.int64)                # node -> position in core list
    for _pass in range(2):
        if _pass == 0:
            part_src = (src // NSH) >= (NC // 2)  # initial estimate: by core
        else:
            part_src = newpos[src] >= PSPLIT
        dl_all = np.bincount(dst[~part_src], minlength=N)
        dh_all = np.bincount(dst[part_src], minlength=N)
        for c in range(NC):
            nodes = np.arange(c * NSH, (c + 1) * NSH)
            dl, dh = dl_all[nodes], dh_all[nodes]
            order_n = np.argsort(-(dl + dh), kind="stable")
            cnt_l = np.zeros(NCHUNK, np.int64)
            cnt_h = np.zeros(NCHUNK, np.int64)
            fill = np.zeros(NCHUNK, np.int64)
            pos_of = np.empty(NSH, np.int64)
            for j in order_n:
                load = np.maximum(cnt_l + dl[j], cnt_h + dh[j]) + (cnt_l + cnt_h) * 1e-3
                load[fill >= P] = 1 << 40
                b = int(np.argmin(load))
                pos_of[j] = b * P + fill[b]
                fill[b] += 1
                cnt_l[b] += dl[j]
                cnt_h[b] += dh[j]
            newpos[nodes] = pos_of

    src_pad = (src // NSH) * NSHP + newpos[src]
    dst_key = (dst // NSH) * NSHP + newpos[dst]   # position-space dst

    order = np.argsort(dst_key, kind="stable")
    dsts = dst_key[order]
    srcs_pad = src_pad[order]
    core_bounds = np.searchsorted(dsts, np.arange(NC + 1) * NSHP)

    # per-core, per-chunk, split into part-1/part-2 table slices (int16
    # index range: NC*NP1 = 25600 rows < 32768)
    metas = []
    S_LO = S_HI = 1
    for c in range(NC):
        lo, hi = core_bounds[c], core_bounds[c + 1]
        d_loc = (dsts[lo:hi] - c * NSHP).astype(np.int64)
        s_pad = srcs_pad[lo:hi]
        chunk = d_loc // P
        half = ((s_pad % NSHP) >= PSPLIT).astype(np.int64)   # src part
        cnt_lo = np.bincount(chunk[half == 0], minlength=NCHUNK)
        cnt_hi = np.bincount(chunk[half == 1], minlength=NCHUNK)
        S_LO = max(S_LO, int(np.ceil(cnt_lo.max() / P)))
        S_HI = max(S_HI, int(np.ceil(cnt_hi.max() / P)))
        metas.append((d_loc, s_pad, chunk, half, cnt_lo, cnt_hi))
    S = S_LO + S_HI

    in_maps = []
    iota = np.broadcast_to(np.arange(P, dtype=np.float32), (P, P)).copy()
    iotat = np.arange(P, dtype=np.float32).reshape(P, 1)
    ident = np.eye(P, dtype=np.float32)
    BD = np.zeros((C1, 2 * HEADS), np.float32)
    for h in range(HEADS):
        BD[h * HID:(h + 1) * HID, h] = a_src1[h]
        BD[h * HID:(h + 1) * HID, HEADS + h] = a_dst1[h]
    A1 = np.concatenate([BD[:P], BD[P:]], axis=1)          # [128, 16]
    W2h = np.concatenate([W2[:P], W2[P:]], axis=1)         # [128, 128]
    A2 = np.concatenate([a_src2.T, a_dst2.T], axis=1)      # [64, 2]
    B1R = np.broadcast_to(b1, (P, C1)).copy()
    B2R = np.broadcast_to(b2, (P, HID)).copy()
    BLR = np.broadcast_to(bl, (P, OUT_CH)).copy()

    import ml_dtypes
    bf16 = ml_dtypes.bfloat16

    nslot = NCHUNK * S * P

    def wrap16(vals):
        # dma_gather index layout: idx j -> [j % 16, j // 16], x8 replicated
        return np.tile(vals.reshape(-1, 16).T, (8, 1))

    for c in range(NC):
        d_loc, s_pad, chunk, half, cnt_lo, cnt_hi = metas[c]
        # slot within chunk: lo edges at [0, S_LO*P), hi at [S_LO*P, S*P)
        start_lo = np.zeros(NCHUNK, np.int64)
        start_lo[1:] = np.cumsum(cnt_lo)[:-1]
        start_hi = np.zeros(NCHUNK, np.int64)
        start_hi[1:] = np.cumsum(cnt_hi)[:-1]
        # rank of each edge within its (chunk, half) group
        ordh = np.lexsort((half, chunk))  # stable by (chunk, half)
        # easier: compute positions via cumcount per (chunk, half)
        key = chunk * 2 + half
        orderk = np.argsort(key, kind="stable")
        ks = key[orderk]
        kstart = np.searchsorted(ks, np.arange(2 * NCHUNK))
        pos_sorted = np.arange(len(ks)) - kstart[ks]
        pos = np.empty(len(ks), np.int64)
        pos[orderk] = pos_sorted
        slot = chunk * (S * P) + np.where(half == 0, pos, S_LO * P + pos)

        idx16 = np.zeros(nslot, np.int16)      # pad slots fetch row 0 (harmless)
        edstl = np.full(nslot, -1.0, np.float32)
        s_core = s_pad // NSHP
        s_pos = s_pad % NSHP
        part_row = np.where(half == 0, s_core * NP1 + s_pos,
                            s_core * NP2 + (s_pos - PSPLIT))
        idx16[slot] = part_row.astype(np.int16)
        edstl[slot] = (d_loc % P).astype(np.float32)

        # int16 wrapped index blocks per chunk: [128, NCHUNK*S*8]
        idx_chunks = idx16.reshape(NCHUNK, S * P)
        idxw = np.concatenate([wrap16(idx_chunks[ch]) for ch in range(NCHUNK)],
                              axis=1)
        edstl_m = np.ascontiguousarray(edstl.reshape(NCHUNK * S, P).T)
        # free-dim (slot-major) layout replicated across partitions, for
        # building transposed one-hots directly on the DVE
        edstlt = np.broadcast_to(edstl.reshape(1, NCHUNK * S * P), (P, NCHUNK * S * P))

        xT = np.zeros((IN_CH, NSHP), np.float32)
        nodes_c = np.arange(c * NSH, (c + 1) * NSH)
        xT[:, newpos[nodes_c]] = x[nodes_c].T

        in_maps.append({
            "xT": xT.astype(bf16), "W1": W1.astype(bf16),
            "A1": A1.astype(bf16), "W2h": W2h.astype(bf16),
            "A2": A2.astype(bf16), "WL": Wl.astype(bf16),
            "B1R": B1R.astype(bf16), "B2R": B2R.astype(bf16),
            "BLR": BLR.astype(bf16), "IOTA": iota.astype(bf16),
            "IOTAT": iotat, "IDENT": ident.astype(bf16),
            "IDXM": np.ascontiguousarray(idxw), "EDSTL": edstl_m,
            "EDSTLT": np.ascontiguousarray(edstlt.astype(np.int8)),
        })
    cfg = dict(cfg)
    cfg["S"] = S
    cfg["S_LO"] = S_LO
    cfg["S_HI"] = S_HI
    cfg["PSPLIT"] = PSPLIT
    cfg["NP1"] = NP1
    cfg["NP2"] = NP2
    cfg["NEWPOS"] = newpos
    return in_maps, cfg


# ---------------------------------------------------------------- device build
def build(cfg, reps=1, stages=("A", "B", "B2", "C")):
    stages = set(stages)
    import concourse.bacc as bacc
    import concourse.bass as bass
    import concourse.mybir as mybir
    import concourse.tile as tile
    from concourse.replica_groups import maybe_share_collective_output_space

    NC, NSH, NSHP, NCHUNK, S = (cfg["NC"], cfg["NSH"], cfg["NSHP"],
                                cfg["NCHUNK"], cfg["S"])
    f32 = mybir.dt.float32
    bf16 = mybir.dt.bfloat16
    i32 = mybir.dt.int32
    AFT = mybir.ActivationFunctionType
    ALU = mybir.AluOpType
    rg = [list(range(NC))]
    share = maybe_share_collective_output_space("AllGather", rg)

    nc = bacc.Bacc("TRN2", target_bir_lowering=False, debug=False,
                   enable_asserts=False, num_devices=NC)

    def ein(name, shape, dt=f32):
        return nc.dram_tensor(name, shape, dt, kind="ExternalInput")

    t_xT = ein("xT", [IN_CH, NSHP], bf16)
    t_W1 = ein("W1", [IN_CH, C1], bf16)
    t_A1 = ein("A1", [P, 4 * HEADS], bf16)
    t_W2h = ein("W2h", [P, 2 * HID], bf16)
    t_A2 = ein("A2", [HID, 2], bf16)
    t_WL = ein("WL", [HID, OUT_CH], bf16)
    t_B1R = ein("B1R", [P, C1], bf16)
    t_B2R = ein("B2R", [P, HID], bf16)
    t_BLR = ein("BLR", [P, OUT_CH], bf16)
    t_IOTA = ein("IOTA", [P, P], bf16)
    t_IOTAT = ein("IOTAT", [P, 1])
    t_IDENT = ein("IDENT", [P, P], bf16)
    S_LO, S_HI = cfg["S_LO"], cfg["S_HI"]
    HALF = NC * NSHP // 2
    t_IDXM = ein("IDXM", [P, NCHUNK * S * 8], mybir.dt.int16)
    t_EDSTL = ein("EDSTL", [P, NCHUNK * S])
    t_EDSTLT = ein("EDSTLT", [P, NCHUNK * S * P], mybir.dt.int8)

    t_Y = nc.dram_tensor("Y", [NSHP, OUT_CH], f32, kind="ExternalOutput")

    with tile.TileContext(nc) as tc:
        with tc.tile_pool(name="dramp", bufs=1, space="DRAM") as dramp, \
             tc.tile_pool(name="consts", bufs=1) as cns, \
             tc.tile_pool(name="h1p", bufs=1) as h1p, \
             tc.tile_pool(name="gp", bufs=2) as gp, \
             tc.tile_pool(name="wk", bufs=3) as wk, \
             tc.tile_pool(name="ps", bufs=1, space="PSUM") as ps:

            # ---- load constants
            def cload(t, shape, dt=f32):
                tl = cns.tile(shape, dt, name=t.name + "_sb")
                nc.sync.dma_start(tl[:], t.ap())
                return tl

            xT_sb = cload(t_xT, [IN_CH, NSHP], bf16)
            W1_sb = cload(t_W1, [IN_CH, C1], bf16)
            A1_sb = cload(t_A1, [P, 4 * HEADS], bf16)
            W2h_sb = cload(t_W2h, [P, 2 * HID], bf16)
            A2_sb = cload(t_A2, [HID, 2], bf16)
            WL_sb = cload(t_WL, [HID, OUT_CH], bf16)
            B1R_sb = cload(t_B1R, [P, C1], bf16)
            B2R_sb = cload(t_B2R, [P, HID], bf16)
            BLR_sb = cload(t_BLR, [P, OUT_CH], bf16)
            IOTA_sb = cload(t_IOTA, [P, P], bf16)
            IOTAT_sb = cload(t_IOTAT, [P, 1])
            IDENT_sb = cload(t_IDENT, [P, P], bf16)
            IDXM_sb = cload(t_IDXM, [P, NCHUNK * S * 8], mybir.dt.int16)
            EDSTL_sb = cload(t_EDSTL, [P, NCHUNK * S])

            h1sh = h1p.tile([P, NCHUNK * C1], bf16)
            adst_sh = h1p.tile([P, NCHUNK * 4], bf16)
            adst2_sh = h1p.tile([P, NCHUNK], bf16)
            # per-edge-slot dst logit terms, precomputed during the AllGathers
            adp_sb = h1p.tile([P, NCHUNK * S * HEADS], f32)
            adp2_sb = h1p.tile([P, NCHUNK * S], f32)
            # part-1 partial aggregates parked between the two passes
            accB = h1p.tile([P, NCHUNK * 4 * (HID + 1)], bf16)
            accC = h1p.tile([P, NCHUNK * (HID + 1)], bf16)

            PSPLIT, NP1, NP2 = cfg["PSPLIT"], cfg["NP1"], cfg["NP2"]
            CSPLIT = PSPLIT // P              # chunks in part 1

            # ===== stages (optionally repeated in-NEFF for timing) =====
            for _rep in range(reps):
              tb1s = dramp.tile([NSHP, ROW1], bf16, tag=f"tb1s_{_rep}", name="tb1s")
              tb1a = dramp.tile([NC * NP1, ROW1], bf16, addr_space=share,
                                tag=f"tb1a_{_rep}", name="tb1a")
              tb1b = dramp.tile([NC * NP2, ROW1], bf16, addr_space=share,
                                tag=f"tb1b_{_rep}", name="tb1b")
              tb2s = dramp.tile([NSHP, ROW2], bf16, tag=f"tb2s_{_rep}", name="tb2s")
              tb2a = dramp.tile([NC * NP1, ROW2], bf16, addr_space=share,
                                tag=f"tb2a_{_rep}", name="tb2a")
              tb2b = dramp.tile([NC * NP2, ROW2], bf16, addr_space=share,
                                tag=f"tb2b_{_rep}", name="tb2b")
              # =================== stage A: node table 1 ===================
              for i in range(NCHUNK if "A" in stages else 0):
                  rx = xT_sb[:, i * P:(i + 1) * P]
                  hp0 = ps.tile([P, P], f32, tag="tr", bufs=4, name="hp0")
                  hp1 = ps.tile([P, P], f32, tag="tr", bufs=4, name="hp1")
                  nc.tensor.matmul(out=hp0[:], lhsT=W1_sb[:, 0:P], rhs=rx,
                                   start=True, stop=True)
                  nc.tensor.matmul(out=hp1[:], lhsT=W1_sb[:, P:C1], rhs=rx,
                                   start=True, stop=True)
                  h0 = wk.tile([P, P], bf16, tag="hc0", name="h0")
                  h1c = wk.tile([P, P], bf16, tag="hc1", name="h1c")
                  nc.vector.tensor_copy(h0[:], hp0[:])
                  nc.vector.tensor_copy(h1c[:], hp1[:])
                  aa = ps.tile([2 * HEADS, P], f32, tag="sm", bufs=2, name="aa")
                  nc.tensor.matmul(out=aa[:], lhsT=A1_sb[:, 0:2 * HEADS],
                                   rhs=h0[:], start=True, stop=False)
                  nc.tensor.matmul(out=aa[:], lhsT=A1_sb[:, 2 * HEADS:4 * HEADS],
                                   rhs=h1c[:], start=False, stop=True)
                  t0 = ps.tile([P, P], bf16, tag="tr", bufs=4, name="t0")
                  t1 = ps.tile([P, P], bf16, tag="tr", bufs=4, name="t1")
                  nc.tensor.transpose(out=t0[:], in_=h0[:], identity=IDENT_sb[:])
                  nc.tensor.transpose(out=t1[:], in_=h1c[:], identity=IDENT_sb[:])
                  aa_sb = wk.tile([2 * HEADS, P], bf16, tag="aa_sb", name="aa_sb")
                  nc.vector.tensor_copy(aa_sb[:], aa[:])
                  aat = ps.tile([P, 2 * HEADS], bf16, tag="sm", bufs=2, name="aat")
                  nc.tensor.transpose(out=aat[:], in_=aa_sb[:],
                                      identity=IDENT_sb[:2 * HEADS, :2 * HEADS])
                  row = wk.tile([P, ROW1], bf16, tag="row", name="row")
                  nc.vector.memset(row[:, 4 * (HID + 1) + 8:ROW1], 0.0)
                  # heads 0,1 from t0; heads 2,3 from t1; interleaved ones cols
                  nc.scalar.copy(row[:, 0:HID], t0[:, 0:HID])
                  nc.scalar.copy(row[:, HID + 1:2 * HID + 1], t0[:, HID:2 * HID])
                  nc.scalar.copy(row[:, 2 * (HID + 1):2 * (HID + 1) + HID],
                                 t1[:, 0:HID])
                  nc.scalar.copy(row[:, 3 * (HID + 1):3 * (HID + 1) + HID],
                                 t1[:, HID:2 * HID])
                  nc.vector.memset(
                      row[:, 0:4 * (HID + 1)].rearrange(
                          "p (h q) -> p h q", h=HEADS)[:, :, HID:HID + 1], 1.0)
                  nc.vector.tensor_copy(row[:, 4 * (HID + 1):4 * (HID + 1) + 8],
                                        aat[:])
                  nc.vector.tensor_copy(adst_sh[:, i * 4:(i + 1) * 4],
                                        aat[:, 4:8])
                  nc.sync.dma_start(tb1s[i * P:(i + 1) * P, :], row[:])
                  if i == CSPLIT - 1:
                      # part-1 rows done: overlap its AllGather with the rest
                      # of stage A and the prep phase
                      nc.gpsimd.collective_compute(
                          "AllGather", ALU.bypass, replica_groups=rg,
                          ins=[tb1s[0:PSPLIT, :]], outs=[tb1a[:]])

              if "A" in stages:
                  nc.gpsimd.collective_compute(
                      "AllGather", ALU.bypass, replica_groups=rg,
                      ins=[tb1s[PSPLIT:NSHP, :]], outs=[tb1b[:]])

              # Manual scheduler phasing: the scheduling-pass sim models the
              # collectives as fast, so without this it interleaves gather-
              # dependent stage-B ops into the prep stream; the in-order
              # engine queues then stall on the first gather instead of
              # running prep during the AllGather. The wait timestamps only
              # steer the scheduling pass, not the real-run semaphores.
              _PH = 10.0 * _rep

              # ========== prep-1 (overlaps AG1): direct transposed one-hots
              # feed adp (per-edge dst logit terms); no gather dependency.
              tc.tile_set_cur_wait(_PH + 1)
              for c in range(NCHUNK if "B" in stages else 0):
                  edt = gp.tile([P, S * P], mybir.dt.int8, tag="edt", name="edt")
                  nc.sync.dma_start(edt[:],
                                    t_EDSTLT.ap()[:, c * S * P:(c + 1) * S * P])
                  adp_ps = ps.tile([P, S * HEADS], f32, tag="sm", bufs=2,
                                   name="adp_ps")
                  for t in range(S):
                      s0t = wk.tile([P, P], bf16, tag="s0t", bufs=4, name="s0t")
                      nc.vector.tensor_scalar(
                          out=s0t[:], in0=edt[:, t * P:(t + 1) * P],
                          scalar1=IOTAT_sb[:, 0:1], scalar2=None,
                          op0=ALU.is_equal)
                      nc.tensor.matmul(out=adp_ps[:, t * HEADS:(t + 1) * HEADS],
                                       lhsT=s0t[:],
                                       rhs=adst_sh[:, c * 4:(c + 1) * 4],
                                       start=True, stop=True)
                  nc.scalar.copy(adp_sb[:, c * S * HEADS:(c + 1) * S * HEADS],
                                 adp_ps[:])

              # =================== stage B: layer-1 edge aggregation =======
              # Two passes, one per table part: each part's gathers/compute
              # overlap the other part's AllGather. Part-1 partial sums park
              # in SBUF (bf16) until part 2 completes the chunk.
              ACOLS = 4 * (HID + 1)      # 260 aggregated columns
              QP = 5                     # subtiles per exp/scale piece

              def emit_logit_mm(c, g, psB, toff, scnt, opened, part):
                  # logit chain: asrc (gathered) + adp (precomputed), then
                  # broadcast-exp on the scalar engine (expanded bf16 alpha so
                  # the row scaling runs in the DVE high-perf mode), pieces so
                  # Act/DVE/PE pipeline within the chunk.
                  elin = wk.tile([P, scnt * HEADS], f32, tag=f"elin_p{part}",
                                 bufs=2, name="elin")
                  nc.vector.tensor_tensor(
                      out=elin[:].rearrange("p (t h) -> p t h", h=HEADS),
                      in0=g[:, :, ACOLS:ACOLS + HEADS],
                      in1=adp_sb[:, (c * S + toff) * HEADS:
                                 (c * S + toff + scnt) * HEADS
                                 ].rearrange("p (t h) -> p t h", h=HEADS),
                      op=ALU.add)
                  elr = wk.tile([P, scnt * HEADS], f32, tag=f"elr_p{part}",
                                bufs=2, name="elr")
                  nc.vector.scalar_tensor_tensor(
                      out=elr[:], in0=elin[:], scalar=NEG_SLOPE, in1=elin[:],
                      op0=ALU.mult, op1=ALU.max)
                  eexpx = wk.tile([P, scnt, ACOLS], bf16, tag=f"eexpx_p{part}",
                                  bufs=1, name="eexpx")
                  rhsa = wk.tile([P, scnt, ACOLS], bf16, tag=f"rhsa_p{part}",
                                 bufs=2, name="rhsa")
                  for q0 in range(0, scnt, QP):
                      q1 = min(q0 + QP, scnt)
                      nc.scalar.activation(
                          eexpx[:, q0:q1, :].rearrange(
                              "p t (h q) -> p t h q", h=HEADS),
                          elr[:, q0 * HEADS:q1 * HEADS].rearrange(
                              "p (t h) -> p t h", h=HEADS)[
                              :, :, :, None].to_broadcast(
                                  [P, q1 - q0, HEADS, HID + 1]),
                          AFT.Exp)
                      nc.vector.tensor_tensor(
                          out=rhsa[:, q0:q1, :], in0=g[:, q0:q1, 0:ACOLS],
                          in1=eexpx[:, q0:q1, :], op=ALU.mult)
                  for t in range(scnt):
                      col = c * S + toff + t
                      s0 = wk.tile([P, P], bf16, tag="s0", bufs=4, name="s0")
                      nc.vector.tensor_scalar(
                          out=s0[:], in0=IOTA_sb[:],
                          scalar1=EDSTL_sb[:, col:col + 1], scalar2=None,
                          op0=ALU.is_equal)
                      nc.tensor.matmul(out=psB[:], lhsT=s0[:],
                                       rhs=rhsa[:, t, :],
                                       start=(not opened and t == 0),
                                       stop=(t == scnt - 1))

              # ---- pass 1: part-1 gathers + self loops, park partials
              tc.tile_set_cur_wait(_PH + 2)
              for c in range(NCHUNK if "B" in stages else 0):
                  g = gp.tile([P, S_LO, ROW1], bf16, tag="gp1", name="gp1")
                  nc.gpsimd.dma_gather(
                      out_ap=g[:], in_ap=tb1a[:],
                      idxs_ap=IDXM_sb[:, c * S * 8:c * S * 8 + S_LO * 8],
                      num_idxs=S_LO * P, num_idxs_reg=S_LO * P,
                      elem_size=ROW1, single_packet=False)
                  psB = ps.tile([P, ACOLS], f32, tag="agg", bufs=2, name="psB")
                  # self-loop contribution: local rows, no gather; the
                  # diagonal matmul (lhsT=I) opens the psB accumulation
                  rself = gp.tile([P, ROW1], bf16, tag="rself", name="rself")
                  nc.sync.dma_start(rself[:], tb1s[c * P:(c + 1) * P, :])
                  elinS = wk.tile([P, HEADS], f32, tag="elinS", bufs=2,
                                  name="elinS")
                  nc.vector.tensor_tensor(
                      out=elinS[:], in0=rself[:, ACOLS:ACOLS + HEADS],
                      in1=adst_sh[:, c * 4:(c + 1) * 4], op=ALU.add)
                  elrS = wk.tile([P, HEADS], f32, tag="elrS", bufs=2,
                                 name="elrS")
                  nc.vector.scalar_tensor_tensor(
                      out=elrS[:], in0=elinS[:], scalar=NEG_SLOPE, in1=elinS[:],
                      op0=ALU.mult, op1=ALU.max)
                  eexpSx = wk.tile([P, ACOLS], bf16, tag="eexpSx", bufs=2,
                                   name="eexpSx")
                  nc.scalar.activation(
                      eexpSx[:].rearrange("p (h q) -> p h q", h=HEADS),
                      elrS[:, :, None].to_broadcast([P, HEADS, HID + 1]),
                      AFT.Exp)
                  rhsaS = wk.tile([P, ACOLS], bf16, tag="rhsaS", bufs=2,
                                  name="rhsaS")
                  nc.vector.tensor_tensor(
                      out=rhsaS[:], in0=rself[:, 0:ACOLS], in1=eexpSx[:],
                      op=ALU.mult)
                  nc.tensor.matmul(out=psB[:], lhsT=IDENT_sb[:], rhs=rhsaS[:],
                                   start=True, stop=False)
                  emit_logit_mm(c, g, psB, 0, S_LO, opened=True, part=1)
                  nc.scalar.copy(accB[:, c * ACOLS:(c + 1) * ACOLS], psB[:])

              # ---- pass 2: part-2 gathers, combine, epilogue, B2
              tc.tile_set_cur_wait(_PH + 3)
              for c in range(NCHUNK if "B" in stages else 0):
                  g = gp.tile([P, S_HI, ROW1], bf16, tag="gp2", name="gp2")
                  nc.gpsimd.dma_gather(
                      out_ap=g[:], in_ap=tb1b[:],
                      idxs_ap=IDXM_sb[:, c * S * 8 + S_LO * 8:(c + 1) * S * 8],
                      num_idxs=S_HI * P, num_idxs_reg=S_HI * P,
                      elem_size=ROW1, single_packet=False)
                  psB = ps.tile([P, ACOLS], f32, tag="agg", bufs=2, name="psB2")
                  emit_logit_mm(c, g, psB, S_LO, S_HI, opened=False, part=2)
                  hsum = wk.tile([P, ACOLS], f32, tag="hsum", bufs=2,
                                 name="hsum")
                  nc.vector.tensor_tensor(
                      out=hsum[:], in0=psB[:],
                      in1=accB[:, c * ACOLS:(c + 1) * ACOLS], op=ALU.add)
                  # epilogue: normalize + bias + relu -> h1 chunk
                  den = wk.tile([P, HEADS], f32, tag="den", name="den")
                  nc.vector.tensor_scalar(
                      out=den[:], in0=hsum[:, HID::HID + 1].to_broadcast([P, HEADS]),
                      scalar1=EPS, scalar2=None, op0=ALU.add)
                  rec = wk.tile([P, HEADS], f32, tag="rec", name="rec")
                  nc.vector.reciprocal(rec[:], den[:])
                  h1n = wk.tile([P, C1], bf16, tag="h1n", name="h1n")
                  nc.vector.tensor_tensor(
                      out=h1n[:].rearrange("p (h q) -> p h q", h=HEADS),
                      in0=hsum[:].rearrange("p (h q) -> p h q", h=HEADS)[:, :, 0:HID],
                      in1=rec[:, :, None].to_broadcast([P, HEADS, HID]),
                      op=ALU.mult)
                  nc.vector.tensor_tensor(out=h1n[:], in0=h1n[:], in1=B1R_sb[:],
                                          op=ALU.add)
                  nc.vector.tensor_scalar(
                      out=h1sh[:, c * C1:(c + 1) * C1], in0=h1n[:],
                      scalar1=0.0, scalar2=None, op0=ALU.max)

                  # ========== stage B' (interleaved): node table 2 ==========
                  if "B2" in stages:
                      i = c
                      h1t = h1sh[:, i * C1:(i + 1) * C1]
                      q0 = ps.tile([P, P], bf16, tag="tr", bufs=4, name="q0")
                      q1 = ps.tile([P, P], bf16, tag="tr", bufs=4, name="q1")
                      nc.tensor.transpose(out=q0[:], in_=h1t[:, 0:P], identity=IDENT_sb[:])
                      nc.tensor.transpose(out=q1[:], in_=h1t[:, P:C1], identity=IDENT_sb[:])
                      ht0 = wk.tile([P, P], bf16, tag="hc0", name="ht0")
                      ht1 = wk.tile([P, P], bf16, tag="hc1", name="ht1")
                      nc.vector.tensor_copy(ht0[:], q0[:])
                      nc.vector.tensor_copy(ht1[:], q1[:])
                      h2p = ps.tile([HID, P], f32, tag="tr", bufs=4, name="h2p")
                      nc.tensor.matmul(out=h2p[:], lhsT=W2h_sb[:, 0:HID], rhs=ht0[:],
                                       start=True, stop=False)
                      nc.tensor.matmul(out=h2p[:], lhsT=W2h_sb[:, HID:2 * HID],
                                       rhs=ht1[:], start=False, stop=True)
                      h2t = wk.tile([HID, P], bf16, tag="h2t", name="h2t")
                      nc.vector.tensor_copy(h2t[:], h2p[:])
                      aa2 = ps.tile([2, P], f32, tag="sm", bufs=2, name="aa2")
                      nc.tensor.matmul(out=aa2[:], lhsT=A2_sb[:], rhs=h2t[:],
                                       start=True, stop=True)
                      aa2_sb = wk.tile([2, P], bf16, tag="aa2_sb", name="aa2_sb")
                      nc.vector.tensor_copy(aa2_sb[:], aa2[:])
                      r2h = ps.tile([P, HID], bf16, tag="tr", bufs=4, name="r2h")
                      nc.tensor.transpose(out=r2h[:], in_=h2t[:], identity=IDENT_sb[:HID, :HID])
                      r2a = ps.tile([P, 2], bf16, tag="sm", bufs=2, name="r2a")
                      nc.tensor.transpose(out=r2a[:], in_=aa2_sb[:], identity=IDENT_sb[:2, :2])
                      row2 = wk.tile([P, ROW2], bf16, tag="row2", name="row2")
                      nc.vector.memset(row2[:, HID + 2:ROW2], 0.0)
                      nc.vector.tensor_copy(row2[:, 0:HID], r2h[:])
                      nc.vector.memset(row2[:, HID:HID + 1], 1.0)
                      nc.vector.tensor_copy(row2[:, HID + 1:HID + 2], r2a[:, 0:1])
                      nc.vector.tensor_copy(adst2_sh[:, i:i + 1], r2a[:, 1:2])
                      nc.sync.dma_start(tb2s[i * P:(i + 1) * P, :], row2[:])
                      if i == CSPLIT - 1:
                          nc.gpsimd.collective_compute(
                              "AllGather", ALU.bypass, replica_groups=rg,
                              ins=[tb2s[0:PSPLIT, :]], outs=[tb2a[:]])

              if "B2" in stages:
                  nc.gpsimd.collective_compute(
                      "AllGather", ALU.bypass, replica_groups=rg,
                      ins=[tb2s[PSPLIT:NSHP, :]], outs=[tb2b[:]])

              # ========== prep-2 (overlaps AG2): adp2 for layer 2 ==========
              tc.tile_set_cur_wait(_PH + 4)
              for c in range(NCHUNK if "C" in stages else 0):
                  edt2 = gp.tile([P, S * P], mybir.dt.int8, tag="edt", name="edt2")
                  nc.sync.dma_start(edt2[:],
                                    t_EDSTLT.ap()[:, c * S * P:(c + 1) * S * P])
                  adp2_ps = ps.tile([P, S], f32, tag="sm", bufs=2,
                                    name="adp2_ps")
                  for t in range(S):
                      s0t2 = wk.tile([P, P], bf16, tag="s0t", bufs=4, name="s0t2")
                      nc.vector.tensor_scalar(
                          out=s0t2[:], in0=edt2[:, t * P:(t + 1) * P],
                          scalar1=IOTAT_sb[:, 0:1], scalar2=None,
                          op0=ALU.is_equal)
                      nc.tensor.matmul(out=adp2_ps[:, t:t + 1], lhsT=s0t2[:],
                                       rhs=adst2_sh[:, c:c + 1],
                                       start=True, stop=True)
                  nc.scalar.copy(adp2_sb[:, c * S:(c + 1) * S], adp2_ps[:])

              if "C" not in stages and "B" in stages:
                  keep = wk.tile([P, OUT_CH], f32, tag="yout", name="keep")
                  nc.vector.tensor_copy(keep[:], h1sh[:, 0:OUT_CH])
                  nc.sync.dma_start(t_Y.ap()[0:P, :], keep[:])

              # =================== stage C: layer-2 aggregation + final ====
              # Same two-pass structure as stage B.
              def emit_c_logit_mm(c, g2, psC, toff, scnt, opened, part):
                  elin2 = wk.tile([P, scnt], f32, tag=f"elin2_p{part}", bufs=2,
                                  name="elin2")
                  nc.vector.tensor_tensor(
                      out=elin2[:, :, None], in0=g2[:, :, HID + 1:HID + 2],
                      in1=adp2_sb[:, c * S + toff:c * S + toff + scnt, None],
                      op=ALU.add)
                  elr2 = wk.tile([P, scnt], f32, tag=f"elr2_p{part}", bufs=2,
                                 name="elr2")
                  nc.vector.scalar_tensor_tensor(
                      out=elr2[:], in0=elin2[:], scalar=NEG_SLOPE, in1=elin2[:],
                      op0=ALU.mult, op1=ALU.max)
                  eexp2 = wk.tile([P, scnt], f32, tag=f"eexp2_p{part}", bufs=2,
                                  name="eexp2")
                  nc.scalar.activation(eexp2[:], elr2[:], AFT.Exp)
                  for t in range(scnt):
                      col = c * S + toff + t
                      s0w = wk.tile([P, P], bf16, tag="s0w", bufs=6, name="s0w")
                      nc.vector.tensor_scalar(
                          out=s0w[:], in0=IOTA_sb[:],
                          scalar1=EDSTL_sb[:, col:col + 1],
                          scalar2=eexp2[:, t:t + 1],
                          op0=ALU.is_equal, op1=ALU.mult)
                      nc.tensor.matmul(out=psC[:], lhsT=s0w[:],
                                       rhs=g2[:, t, 0:HID + 1],
                                       start=(not opened and t == 0),
                                       stop=(t == scnt - 1))

              # ---- pass 1: part-1 gathers + self loops, park partials
              tc.tile_set_cur_wait(_PH + 5)
              for c in range(NCHUNK if "C" in stages else 0):
                  g2 = gp.tile([P, S_LO, ROW2], bf16, tag="g2p1", bufs=3,
                               name="g2p1")
                  nc.gpsimd.dma_gather(
                      out_ap=g2[:], in_ap=tb2a[:],
                      idxs_ap=IDXM_sb[:, c * S * 8:c * S * 8 + S_LO * 8],
                      num_idxs=S_LO * P, num_idxs_reg=S_LO * P,
                      elem_size=ROW2, single_packet=False)
                  psC = ps.tile([P, HID + 1], f32, tag="agg", bufs=2, name="psC")
                  # self-loop contribution from local rows (no gather)
                  rself2 = gp.tile([P, ROW2], bf16, tag="rself2", name="rself2")
                  nc.sync.dma_start(rself2[:], tb2s[c * P:(c + 1) * P, :])
                  elinS2 = wk.tile([P, 1], f32, tag="elinS", bufs=2,
                                   name="elinS2")
                  nc.vector.tensor_tensor(
                      out=elinS2[:], in0=rself2[:, HID + 1:HID + 2],
                      in1=adst2_sh[:, c:c + 1], op=ALU.add)
                  elrS2 = wk.tile([P, 1], f32, tag="elrS", bufs=2, name="elrS2")
                  nc.vector.scalar_tensor_tensor(
                      out=elrS2[:], in0=elinS2[:], scalar=NEG_SLOPE,
                      in1=elinS2[:], op0=ALU.mult, op1=ALU.max)
                  eexpS2 = wk.tile([P, 1], f32, tag="eexpS2", bufs=2,
                                   name="eexpS2")
                  nc.scalar.activation(eexpS2[:], elrS2[:], AFT.Exp)
                  s0S = wk.tile([P, P], bf16, tag="s0w", bufs=6, name="s0S")
                  nc.vector.tensor_scalar(
                      out=s0S[:], in0=IDENT_sb[:], scalar1=eexpS2[:, 0:1],
                      scalar2=None, op0=ALU.mult)
                  nc.tensor.matmul(out=psC[:], lhsT=s0S[:],
                                   rhs=rself2[:, 0:HID + 1],
                                   start=True, stop=False)
                  emit_c_logit_mm(c, g2, psC, 0, S_LO, opened=True, part=1)
                  nc.scalar.copy(accC[:, c * (HID + 1):(c + 1) * (HID + 1)],
                                 psC[:])

              # ---- pass 2: part-2 gathers, combine, epilogue, final linear
              tc.tile_set_cur_wait(_PH + 6)
              for c in range(NCHUNK if "C" in stages else 0):
                  g2 = gp.tile([P, S_HI, ROW2], bf16, tag="g2p2", bufs=3,
                               name="g2p2")
                  nc.gpsimd.dma_gather(
                      out_ap=g2[:], in_ap=tb2b[:],
                      idxs_ap=IDXM_sb[:, c * S * 8 + S_LO * 8:(c + 1) * S * 8],
                      num_idxs=S_HI * P, num_idxs_reg=S_HI * P,
                      elem_size=ROW2, single_packet=False)
                  psC = ps.tile([P, HID + 1], f32, tag="agg", bufs=2,
                                name="psC2")
                  emit_c_logit_mm(c, g2, psC, S_LO, S_HI, opened=False, part=2)
                  hsum2 = wk.tile([P, HID + 1], f32, tag="hsum2", bufs=2,
                                  name="hsum2")
                  nc.vector.tensor_tensor(
                      out=hsum2[:], in0=psC[:],
                      in1=accC[:, c * (HID + 1):(c + 1) * (HID + 1)],
                      op=ALU.add)
                  den2 = wk.tile([P, 1], f32, tag="den", name="den2")
                  nc.vector.tensor_scalar(out=den2[:], in0=hsum2[:, HID:HID + 1],
                                          scalar1=EPS, scalar2=None, op0=ALU.add)
                  rec2 = wk.tile([P, 1], f32, tag="rec", name="rec2")
                  nc.vector.reciprocal(rec2[:], den2[:])
                  h2n = wk.tile([P, HID], bf16, tag="h1n", name="h2n")
                  nc.vector.tensor_scalar(out=h2n[:], in0=hsum2[:, 0:HID],
                                          scalar1=rec2[:, 0:1], scalar2=None,
                                          op0=ALU.mult)
                  nc.vector.tensor_tensor(out=h2n[:], in0=h2n[:], in1=B2R_sb[:],
                                          op=ALU.add)
                  th2 = ps.tile([HID, P], bf16, tag="tr", bufs=4, name="th2")
                  nc.tensor.transpose(out=th2[:], in_=h2n[:],
                                      identity=IDENT_sb[:])
                  th2s = wk.tile([HID, P], bf16, tag="h2t", name="th2s")
                  nc.scalar.copy(th2s[:], th2[:])
                  yo = ps.tile([P, OUT_CH], f32, tag="tr", bufs=4, name="yo")
                  nc.tensor.matmul(out=yo[:], lhsT=th2s[:], rhs=WL_sb[:],
                                   start=True, stop=True)
                  yout = wk.tile([P, OUT_CH], f32, tag="yout", name="yout")
                  nc.vector.tensor_tensor(out=yout[:], in0=yo[:], in1=BLR_sb[:],
                                          op=ALU.add)
                  nc.sync.dma_start(t_Y.ap()[c * P:(c + 1) * P, :], yout[:])

    nc.compile()
    return nc


# ---------------------------------------------------------------- runner
class Runner:
    """Cached PJRT runner: jits once per compiled nc, keeps inputs
    device-resident. Much faster than run_bass_kernel_spmd for repeat calls
    and lets wall-clock approximate device exec time."""

    def __init__(self, nc, n_cores):
        import jax
        from jax.sharding import Mesh, PartitionSpec
        from jax.experimental.shard_map import shard_map
        import concourse.mybir as mybir
        from concourse import bass2jax
        self._jax = jax
        bass2jax.install_neuronx_cc_hook()
        partition_name = (nc.partition_id_tensor.name
                          if nc.partition_id_tensor else None)
        dbg_name = nc.dbg_addr.name if nc.dbg_addr else None
        in_names, out_names, out_avals, zero_outs = [], [], [], []
        for alloc in nc.m.functions[0].allocations:
            if not isinstance(alloc, mybir.MemoryLocationSet):
                continue
            name = alloc.memorylocations[0].name
            if alloc.kind == "ExternalInput":
                if name not in (partition_name, dbg_name):
                    in_names.append(name)
            elif alloc.kind == "ExternalOutput":
                out_names.append(name)
                shape = tuple(alloc.tensor_shape)
                dtype = mybir.dt.np(alloc.dtype)
                out_avals.append(jax.core.ShapedArray(shape, dtype))
                zero_outs.append(np.zeros(shape, dtype))
        self.n_cores = n_cores
        self.in_names = in_names
        self.out_names = out_names
        self.out_avals = out_avals
        self.zero_outs = zero_outs
        n_params = len(in_names)
        n_outs = len(out_names)
        all_in = list(in_names) + list(out_names)
        if dbg_name is not None:
            all_in.append(dbg_name)
        if partition_name is not None:
            all_in.append(partition_name)

        def _body(*args):
            operands = list(args)
            if dbg_name is not None:
                operands.append(jax.numpy.zeros((1, 2), jax.numpy.uint32))
            if partition_name is not None:
                operands.append(bass2jax.partition_id_tensor())
            return tuple(bass2jax._bass_exec_p.bind(
                *operands, out_avals=tuple(out_avals), in_names=tuple(all_in),
                out_names=tuple(out_names), lowering_input_output_aliases=(),
                sim_require_finite=True, sim_require_nnan=True, nc=nc))

        devices = jax.devices()[:n_cores]
        assert len(devices) == n_cores
        if n_cores == 1:
            self.fn = jax.jit(_body, keep_unused=True)
        else:
            mesh = Mesh(np.asarray(devices), ("core",))
            in_specs = (PartitionSpec("core"),) * (n_params + n_outs)
            out_specs = (PartitionSpec("core"),) * n_outs
            self.fn = jax.jit(
                shard_map(_body, mesh=mesh, in_specs=in_specs,
                          out_specs=out_specs, check_rep=False),
                keep_unused=True)
        self._dev_in = None

    def set_inputs(self, in_maps):
        jax = self._jax
        per_core = [[np.asarray(m[n]) for n in self.in_names] for m in in_maps]
        n_params = len(self.in_names)
        if self.n_cores == 1:
            arrs = [per_core[0][i] for i in range(n_params)]
            zer = list(self.zero_outs)
        else:
            arrs = [np.concatenate([per_core[c][i] for c in range(self.n_cores)],
                                   axis=0) for i in range(n_params)]
            zer = [np.zeros((self.n_cores * z.shape[0], *z.shape[1:]), z.dtype)
                   for z in self.zero_outs]
        self._dev_in = [jax.device_put(a) for a in arrs + zer]

    def run(self):
        outs = self.fn(*self._dev_in)
        self._jax.block_until_ready(outs)
        return outs

    def results(self, outs):
        res = []
        for c in range(self.n_cores):
            d = {}
            for i, name in enumerate(self.out_names):
                a = np.asarray(outs[i])
                if self.n_cores > 1:
                    a = a.reshape(self.n_cores, *self.out_avals[i].shape)[c]
                d[name] = a
            res.append(d)
        return res



_CACHE = {}


def _get_built(cfg_key, cfg):
    if cfg_key not in _CACHE:
        _CACHE[cfg_key] = build(cfg)
    return _CACHE[cfg_key]


def _get_runner(cfg, reps=1):
    key = ("runner", cfg["S"], cfg["S_LO"], reps)
    if key not in _CACHE:
        nc = build(cfg, reps=reps)
        _CACHE[key] = Runner(nc, cfg["NC"])
    return _CACHE[key]


def kernel(**inputs) -> np.ndarray:
    cfg = _cfg_full()
    in_maps, cfg = host_prep(inputs, cfg)
    r = _get_runner(cfg)
    r.set_inputs(in_maps)
    res = r.results(r.run())
    NSH = cfg["NSH"]
    newpos = cfg["NEWPOS"]
    y = np.empty((cfg["NC"] * NSH, OUT_CH), np.float32)
    for c in range(cfg["NC"]):
        nodes_c = np.arange(c * NSH, (c + 1) * NSH)
        y[nodes_c] = res[c]["Y"][newpos[nodes_c]]
    return np.ascontiguousarray(y, dtype=np.float32)


if __name__ == "__main__":
    import reference as R
    inp = R.setup_inputs()
    out = kernel(**{k: np.asarray(v) for k, v in inp.items()})
    exp = np.asarray(R.reference(**inp))
    err = np.abs(out - exp).max() / (np.abs(exp).max() + 1e-12)
    print("rel err:", err)



# revision 39
# speedup vs baseline: 1.2198x; 1.2198x over previous
"""GAT (2-layer, 4-head + linear) Bass kernel for 8 Trainium2 NeuronCores.

Strategy (dst-sharded edge parallelism):
  - Nodes are sharded 8 ways. Each core owns all edges whose DESTINATION is
    in its shard, so segment softmax stats and message aggregation need no
    cross-core reduction.
  - Dense projections are data-parallel over node shards; per-node tables
    [features | softmax-logit terms] are AllGathered so every core can
    gather rows for arbitrary source nodes with indirect DMA.
  - Edge aggregation: edges are sorted by dst and grouped into chunks of 128
    destination nodes. Per 128-edge subtile a one-hot selection matrix S0
    (S0[e,d] = [dst_local(e)==d]) is built on the DVE; the TensorEngine then
    computes PSUM[d,:] += S0.T @ (gathered_rows * exp(leaky_relu(logit))),
    accumulating messages AND softmax denominators (via interleaved
    all-ones columns in the node table) in one matmul per subtile.
  - Tables, gathers and matmuls run in bf16 (PSUM accumulation, softmax
    logits and the output stay fp32); tolerance is 2e-2, measured ~4e-3.
  - Host-side rebalancing: each core's node list order (== chunk
    membership) is permuted so per-chunk edge counts are even across both
    src table halves, minimizing the padded subtile count S.

Self-contained: only needs numpy/ml_dtypes + the concourse (Bass) stack.
"""
import math
import numpy as np

# ---- problem dims (hardcoded per spec nn_GAT_27212912788335) ----
N_NODES = 50000
IN_CH = 128
HID = 64
HEADS = 4
OUT_CH = 64
NEG_SLOPE = 0.2
EPS = 1e-16
NCORES = 8
P = 128

C1 = HEADS * HID          # 256
# bf16 table rows; dma_gather requires elem bytes % 256 == 0
ROW1 = 384   # 4*(64 msg + 1 one) + asrc(4) + adst(4) + pad -> 768B rows
ROW2 = 128   # 64 msg + 1 one + asrc2 + pad -> 256B rows


def _cfg_full():
    NSH = N_NODES // NCORES               # 6250
    NCHUNK = math.ceil(NSH / P)           # 49
    NSHP = NCHUNK * P                     # 6272
    return dict(NC=NCORES, NSH=NSH, NSHP=NSHP, NCHUNK=NCHUNK)


# ---------------------------------------------------------------- host prep
def host_prep(inputs, cfg):
    """Build per-core input maps. Only layout/index work on the host — all
    floating-point math happens on device."""
    NC, NSH, NSHP, NCHUNK = cfg["NC"], cfg["NSH"], cfg["NSHP"], cfg["NCHUNK"]
    N = NC * NSH
    HALF = NC * NSHP // 2

    x = np.ascontiguousarray(np.asarray(inputs["x"], dtype=np.float32))
    ei = np.asarray(inputs["edge_index"], dtype=np.int64)
    W1 = np.asarray(inputs["W1"], dtype=np.float32)
    a_src1 = np.asarray(inputs["a_src1"], dtype=np.float32)
    a_dst1 = np.asarray(inputs["a_dst1"], dtype=np.float32)
    b1 = np.asarray(inputs["b1"], dtype=np.float32)
    W2 = np.asarray(inputs["W2"], dtype=np.float32)
    a_src2 = np.asarray(inputs["a_src2"], dtype=np.float32)
    a_dst2 = np.asarray(inputs["a_dst2"], dtype=np.float32)
    b2 = np.asarray(inputs["b2"], dtype=np.float32)
    Wl = np.asarray(inputs["Wl"], dtype=np.float32)
    bl = np.asarray(inputs["bl"], dtype=np.float32)

    # the reference appends one self loop per node; those are handled
    # analytically on-device (diagonal contribution from local rows), so
    # only the real edges go through the gather path
    src = ei[0].copy()
    dst = ei[1].copy()

    # ---- rebalance: reorder each core's node list so per-chunk edge counts
    # (split by src table half) are even. Chunk membership is just list
    # order, so this is free: xT columns, table rows, gather indices and the
    # final Y unshard all use the same host-chosen permutation.
    half_src = (src // NSH) >= (NC // 2)          # src table half (by core)
    dl_all = np.bincount(dst[~half_src], minlength=N)
    dh_all = np.bincount(dst[half_src], minlength=N)
    newpos = np.empty(N, np.int64)                # node -> position in core list
    for c in range(NC):
        nodes = np.arange(c * NSH, (c + 1) * NSH)
        dl, dh = dl_all[nodes], dh_all[nodes]
        order_n = np.argsort(-(dl + dh), kind="stable")
        cnt_l = np.zeros(NCHUNK, np.int64)
        cnt_h = np.zeros(NCHUNK, np.int64)
        fill = np.zeros(NCHUNK, np.int64)
        pos_of = np.empty(NSH, np.int64)
        for j in order_n:
            load = np.maximum(cnt_l + dl[j], cnt_h + dh[j]) + (cnt_l + cnt_h) * 1e-3
            load[fill >= P] = 1 << 40
            b = int(np.argmin(load))
            pos_of[j] = b * P + fill[b]
            fill[b] += 1
            cnt_l[b] += dl[j]
            cnt_h[b] += dh[j]
        newpos[nodes] = pos_of

    src_pad = (src // NSH) * NSHP + newpos[src]
    dst_key = (dst // NSH) * NSHP + newpos[dst]   # position-space dst

    order = np.argsort(dst_key, kind="stable")
    dsts = dst_key[order]
    srcs_pad = src_pad[order]
    core_bounds = np.searchsorted(dsts, np.arange(NC + 1) * NSHP)

    # per-core, per-chunk, split into lo/hi table halves (int16 index range)
    metas = []
    S_LO = S_HI = 1
    for c in range(NC):
        lo, hi = core_bounds[c], core_bounds[c + 1]
        d_loc = (dsts[lo:hi] - c * NSHP).astype(np.int64)
        s_pad = srcs_pad[lo:hi]
        chunk = d_loc // P
        half = (s_pad >= HALF).astype(np.int64)
        cnt_lo = np.bincount(chunk[half == 0], minlength=NCHUNK)
        cnt_hi = np.bincount(chunk[half == 1], minlength=NCHUNK)
        S_LO = max(S_LO, int(np.ceil(cnt_lo.max() / P)))
        S_HI = max(S_HI, int(np.ceil(cnt_hi.max() / P)))
        metas.append((d_loc, s_pad, chunk, half, cnt_lo, cnt_hi))
    S = S_LO + S_HI

    in_maps = []
    iota = np.broadcast_to(np.arange(P, dtype=np.float32), (P, P)).copy()
    iotat = np.arange(P, dtype=np.float32).reshape(P, 1)
    ident = np.eye(P, dtype=np.float32)
    BD = np.zeros((C1, 2 * HEADS), np.float32)
    for h in range(HEADS):
        BD[h * HID:(h + 1) * HID, h] = a_src1[h]
        BD[h * HID:(h + 1) * HID, HEADS + h] = a_dst1[h]
    A1 = np.concatenate([BD[:P], BD[P:]], axis=1)          # [128, 16]
    W2h = np.concatenate([W2[:P], W2[P:]], axis=1)         # [128, 128]
    A2 = np.concatenate([a_src2.T, a_dst2.T], axis=1)      # [64, 2]
    B1R = np.broadcast_to(b1, (P, C1)).copy()
    B2R = np.broadcast_to(b2, (P, HID)).copy()
    BLR = np.broadcast_to(bl, (P, OUT_CH)).copy()

    import ml_dtypes
    bf16 = ml_dtypes.bfloat16

    nslot = NCHUNK * S * P

    def wrap16(vals):
        # dma_gather index layout: idx j -> [j % 16, j // 16], x8 replicated
        return np.tile(vals.reshape(-1, 16).T, (8, 1))

    for c in range(NC):
        d_loc, s_pad, chunk, half, cnt_lo, cnt_hi = metas[c]
        # slot within chunk: lo edges at [0, S_LO*P), hi at [S_LO*P, S*P)
        start_lo = np.zeros(NCHUNK, np.int64)
        start_lo[1:] = np.cumsum(cnt_lo)[:-1]
        start_hi = np.zeros(NCHUNK, np.int64)
        start_hi[1:] = np.cumsum(cnt_hi)[:-1]
        # rank of each edge within its (chunk, half) group
        ordh = np.lexsort((half, chunk))  # stable by (chunk, half)
        # easier: compute positions via cumcount per (chunk, half)
        key = chunk * 2 + half
        orderk = np.argsort(key, kind="stable")
        ks = key[orderk]
        kstart = np.searchsorted(ks, np.arange(2 * NCHUNK))
        pos_sorted = np.arange(len(ks)) - kstart[ks]
        pos = np.empty(len(ks), np.int64)
        pos[orderk] = pos_sorted
        slot = chunk * (S * P) + np.where(half == 0, pos, S_LO * P + pos)

        idx16 = np.zeros(nslot, np.int16)      # pad slots fetch row 0 (harmless)
        edstl = np.full(nslot, -1.0, np.float32)
        idx16[slot] = (s_pad - half * HALF).astype(np.int16)
        edstl[slot] = (d_loc % P).astype(np.float32)

        # int16 wrapped index blocks per chunk: [128, NCHUNK*S*8]
        idx_chunks = idx16.reshape(NCHUNK, S * P)
        idxw = np.concatenate([wrap16(idx_chunks[ch]) for ch in range(NCHUNK)],
                              axis=1)
        edstl_m = np.ascontiguousarray(edstl.reshape(NCHUNK * S, P).T)
        # free-dim (slot-major) layout replicated across partitions, for
        # building transposed one-hots directly on the DVE
        edstlt = np.broadcast_to(edstl.reshape(1, NCHUNK * S * P), (P, NCHUNK * S * P))

        xT = np.zeros((IN_CH, NSHP), np.float32)
        nodes_c = np.arange(c * NSH, (c + 1) * NSH)
        xT[:, newpos[nodes_c]] = x[nodes_c].T

        in_maps.append({
            "xT": xT.astype(bf16), "W1": W1.astype(bf16),
            "A1": A1.astype(bf16), "W2h": W2h.astype(bf16),
            "A2": A2.astype(bf16), "WL": Wl.astype(bf16),
            "B1R": B1R.astype(bf16), "B2R": B2R.astype(bf16),
            "BLR": BLR.astype(bf16), "IOTA": iota.astype(bf16),
            "IOTAT": iotat, "IDENT": ident.astype(bf16),
            "IDXM": np.ascontiguousarray(idxw), "EDSTL": edstl_m,
            "EDSTLT": np.ascontiguousarray(edstlt.astype(np.int8)),
        })
    cfg = dict(cfg)
    cfg["S"] = S
    cfg["S_LO"] = S_LO
    cfg["S_HI"] = S_HI
    cfg["NEWPOS"] = newpos
    return in_maps, cfg


# ---------------------------------------------------------------- device build
def build(cfg, reps=1, stages=("A", "B", "B2", "C")):
    stages = set(stages)
    import concourse.bacc as bacc
    import concourse.bass as bass
    import concourse.mybir as mybir
    import concourse.tile as tile
    from concourse.replica_groups import maybe_share_collective_output_space

    NC, NSH, NSHP, NCHUNK, S = (cfg["NC"], cfg["NSH"], cfg["NSHP"],
                                cfg["NCHUNK"], cfg["S"])
    f32 = mybir.dt.float32
    bf16 = mybir.dt.bfloat16
    i32 = mybir.dt.int32
    AFT = mybir.ActivationFunctionType
    ALU = mybir.AluOpType
    rg = [list(range(NC))]
    share = maybe_share_collective_output_space("AllGather", rg)

    nc = bacc.Bacc("TRN2", target_bir_lowering=False, debug=False,
                   enable_asserts=False, num_devices=NC)

    def ein(name, shape, dt=f32):
        return nc.dram_tensor(name, shape, dt, kind="ExternalInput")

    t_xT = ein("xT", [IN_CH, NSHP], bf16)
    t_W1 = ein("W1", [IN_CH, C1], bf16)
    t_A1 = ein("A1", [P, 4 * HEADS], bf16)
    t_W2h = ein("W2h", [P, 2 * HID], bf16)
    t_A2 = ein("A2", [HID, 2], bf16)
    t_WL = ein("WL", [HID, OUT_CH], bf16)
    t_B1R = ein("B1R", [P, C1], bf16)
    t_B2R = ein("B2R", [P, HID], bf16)
    t_BLR = ein("BLR", [P, OUT_CH], bf16)
    t_IOTA = ein("IOTA", [P, P], bf16)
    t_IOTAT = ein("IOTAT", [P, 1])
    t_IDENT = ein("IDENT", [P, P], bf16)
    S_LO, S_HI = cfg["S_LO"], cfg["S_HI"]
    HALF = NC * NSHP // 2
    t_IDXM = ein("IDXM", [P, NCHUNK * S * 8], mybir.dt.int16)
    t_EDSTL = ein("EDSTL", [P, NCHUNK * S])
    t_EDSTLT = ein("EDSTLT", [P, NCHUNK * S * P], mybir.dt.int8)

    t_Y = nc.dram_tensor("Y", [NSHP, OUT_CH], f32, kind="ExternalOutput")

    with tile.TileContext(nc) as tc:
        with tc.tile_pool(name="dramp", bufs=1, space="DRAM") as dramp, \
             tc.tile_pool(name="consts", bufs=1) as cns, \
             tc.tile_pool(name="h1p", bufs=1) as h1p, \
             tc.tile_pool(name="gp", bufs=2) as gp, \
             tc.tile_pool(name="wk", bufs=3) as wk, \
             tc.tile_pool(name="ps", bufs=1, space="PSUM") as ps:

            # ---- load constants
            def cload(t, shape, dt=f32):
                tl = cns.tile(shape, dt, name=t.name + "_sb")
                nc.sync.dma_start(tl[:], t.ap())
                return tl

            xT_sb = cload(t_xT, [IN_CH, NSHP], bf16)
            W1_sb = cload(t_W1, [IN_CH, C1], bf16)
            A1_sb = cload(t_A1, [P, 4 * HEADS], bf16)
            W2h_sb = cload(t_W2h, [P, 2 * HID], bf16)
            A2_sb = cload(t_A2, [HID, 2], bf16)
            WL_sb = cload(t_WL, [HID, OUT_CH], bf16)
            B1R_sb = cload(t_B1R, [P, C1], bf16)
            B2R_sb = cload(t_B2R, [P, HID], bf16)
            BLR_sb = cload(t_BLR, [P, OUT_CH], bf16)
            IOTA_sb = cload(t_IOTA, [P, P], bf16)
            IOTAT_sb = cload(t_IOTAT, [P, 1])
            IDENT_sb = cload(t_IDENT, [P, P], bf16)
            IDXM_sb = cload(t_IDXM, [P, NCHUNK * S * 8], mybir.dt.int16)
            EDSTL_sb = cload(t_EDSTL, [P, NCHUNK * S])

            h1sh = h1p.tile([P, NCHUNK * C1], bf16)
            adst_sh = h1p.tile([P, NCHUNK * 4], bf16)
            adst2_sh = h1p.tile([P, NCHUNK], bf16)
            # per-edge-slot dst logit terms, precomputed during the AllGathers
            adp_sb = h1p.tile([P, NCHUNK * S * HEADS], f32)
            adp2_sb = h1p.tile([P, NCHUNK * S], f32)

            # ===== stages (optionally repeated in-NEFF for timing) =====
            for _rep in range(reps):
              tb1s = dramp.tile([NSHP, ROW1], bf16, tag=f"tb1s_{_rep}", name="tb1s")
              tb1 = dramp.tile([NC * NSHP, ROW1], bf16, addr_space=share,
                               tag=f"tb1_{_rep}", name="tb1")
              tb2s = dramp.tile([NSHP, ROW2], bf16, tag=f"tb2s_{_rep}", name="tb2s")
              tb2 = dramp.tile([NC * NSHP, ROW2], bf16, addr_space=share,
                               tag=f"tb2_{_rep}", name="tb2")
              # =================== stage A: node table 1 ===================
              for i in range(NCHUNK if "A" in stages else 0):
                  rx = xT_sb[:, i * P:(i + 1) * P]
                  hp0 = ps.tile([P, P], f32, tag="tr", bufs=4, name="hp0")
                  hp1 = ps.tile([P, P], f32, tag="tr", bufs=4, name="hp1")
                  nc.tensor.matmul(out=hp0[:], lhsT=W1_sb[:, 0:P], rhs=rx,
                                   start=True, stop=True)
                  nc.tensor.matmul(out=hp1[:], lhsT=W1_sb[:, P:C1], rhs=rx,
                                   start=True, stop=True)
                  h0 = wk.tile([P, P], bf16, tag="hc0", name="h0")
                  h1c = wk.tile([P, P], bf16, tag="hc1", name="h1c")
                  nc.vector.tensor_copy(h0[:], hp0[:])
                  nc.vector.tensor_copy(h1c[:], hp1[:])
                  aa = ps.tile([2 * HEADS, P], f32, tag="sm", bufs=2, name="aa")
                  nc.tensor.matmul(out=aa[:], lhsT=A1_sb[:, 0:2 * HEADS],
                                   rhs=h0[:], start=True, stop=False)
                  nc.tensor.matmul(out=aa[:], lhsT=A1_sb[:, 2 * HEADS:4 * HEADS],
                                   rhs=h1c[:], start=False, stop=True)
                  t0 = ps.tile([P, P], bf16, tag="tr", bufs=4, name="t0")
                  t1 = ps.tile([P, P], bf16, tag="tr", bufs=4, name="t1")
                  nc.tensor.transpose(out=t0[:], in_=h0[:], identity=IDENT_sb[:])
                  nc.tensor.transpose(out=t1[:], in_=h1c[:], identity=IDENT_sb[:])
                  aa_sb = wk.tile([2 * HEADS, P], bf16, tag="aa_sb", name="aa_sb")
                  nc.vector.tensor_copy(aa_sb[:], aa[:])
                  aat = ps.tile([P, 2 * HEADS], bf16, tag="sm", bufs=2, name="aat")
                  nc.tensor.transpose(out=aat[:], in_=aa_sb[:],
                                      identity=IDENT_sb[:2 * HEADS, :2 * HEADS])
                  row = wk.tile([P, ROW1], bf16, tag="row", name="row")
                  nc.vector.memset(row[:, 4 * (HID + 1) + 8:ROW1], 0.0)
                  # heads 0,1 from t0; heads 2,3 from t1; interleaved ones cols
                  nc.scalar.copy(row[:, 0:HID], t0[:, 0:HID])
                  nc.scalar.copy(row[:, HID + 1:2 * HID + 1], t0[:, HID:2 * HID])
                  nc.scalar.copy(row[:, 2 * (HID + 1):2 * (HID + 1) + HID],
                                 t1[:, 0:HID])
                  nc.scalar.copy(row[:, 3 * (HID + 1):3 * (HID + 1) + HID],
                                 t1[:, HID:2 * HID])
                  nc.vector.memset(
                      row[:, 0:4 * (HID + 1)].rearrange(
                          "p (h q) -> p h q", h=HEADS)[:, :, HID:HID + 1], 1.0)
                  nc.vector.tensor_copy(row[:, 4 * (HID + 1):4 * (HID + 1) + 8],
                                        aat[:])
                  nc.vector.tensor_copy(adst_sh[:, i * 4:(i + 1) * 4],
                                        aat[:, 4:8])
                  nc.sync.dma_start(tb1s[i * P:(i + 1) * P, :], row[:])

              if "A" in stages:
                  nc.gpsimd.collective_compute(
                      "AllGather", ALU.bypass, replica_groups=rg,
                      ins=[tb1s[:]], outs=[tb1[:]])

              # Manual scheduler phasing: the scheduling-pass sim models the
              # collectives as fast, so without this it interleaves gather-
              # dependent stage-B ops into the prep stream; the in-order
              # engine queues then stall on the first gather instead of
              # running prep during the AllGather. The wait timestamps only
              # steer the scheduling pass, not the real-run semaphores.
              _PH = 10.0 * _rep

              # ========== prep-1 (overlaps AG1): direct transposed one-hots
              # feed adp (per-edge dst logit terms); no gather dependency.
              tc.tile_set_cur_wait(_PH + 1)
              for c in range(NCHUNK if "B" in stages else 0):
                  edt = gp.tile([P, S * P], mybir.dt.int8, tag="edt", name="edt")
                  nc.sync.dma_start(edt[:],
                                    t_EDSTLT.ap()[:, c * S * P:(c + 1) * S * P])
                  adp_ps = ps.tile([P, S * HEADS], f32, tag="sm", bufs=2,
                                   name="adp_ps")
                  for t in range(S):
                      s0t = wk.tile([P, P], bf16, tag="s0t", bufs=4, name="s0t")
                      nc.vector.tensor_scalar(
                          out=s0t[:], in0=edt[:, t * P:(t + 1) * P],
                          scalar1=IOTAT_sb[:, 0:1], scalar2=None,
                          op0=ALU.is_equal)
                      nc.tensor.matmul(out=adp_ps[:, t * HEADS:(t + 1) * HEADS],
                                       lhsT=s0t[:],
                                       rhs=adst_sh[:, c * 4:(c + 1) * 4],
                                       start=True, stop=True)
                  nc.scalar.copy(adp_sb[:, c * S * HEADS:(c + 1) * S * HEADS],
                                 adp_ps[:])

              # =================== stage B: layer-1 edge aggregation =======
              ACOLS = 4 * (HID + 1)      # 260 aggregated columns
              tc.tile_set_cur_wait(_PH + 2)
              for c in range(NCHUNK if "B" in stages else 0):
                  g = gp.tile([P, S, ROW1], bf16, tag="g", name="g")
                  nc.gpsimd.dma_gather(
                      out_ap=g[:, 0:S_LO, :], in_ap=tb1[0:HALF, :],
                      idxs_ap=IDXM_sb[:, c * S * 8:c * S * 8 + S_LO * 8],
                      num_idxs=S_LO * P, num_idxs_reg=S_LO * P,
                      elem_size=ROW1, single_packet=False)
                  nc.gpsimd.dma_gather(
                      out_ap=g[:, S_LO:S, :], in_ap=tb1[HALF:2 * HALF, :],
                      idxs_ap=IDXM_sb[:, c * S * 8 + S_LO * 8:(c + 1) * S * 8],
                      num_idxs=S_HI * P, num_idxs_reg=S_HI * P,
                      elem_size=ROW1, single_packet=False)
                  psB = ps.tile([P, ACOLS], f32, tag="agg", bufs=2, name="psB")
                  # self-loop contribution: local rows, no gather; the
                  # diagonal matmul (lhsT=I) opens the psB accumulation
                  rself = gp.tile([P, ROW1], bf16, tag="rself", name="rself")
                  nc.sync.dma_start(rself[:], tb1s[c * P:(c + 1) * P, :])
                  elinS = wk.tile([P, HEADS], f32, tag="elinS", bufs=2,
                                  name="elinS")
                  nc.vector.tensor_tensor(
                      out=elinS[:], in0=rself[:, ACOLS:ACOLS + HEADS],
                      in1=adst_sh[:, c * 4:(c + 1) * 4], op=ALU.add)
                  elrS = wk.tile([P, HEADS], f32, tag="elrS", bufs=2,
                                 name="elrS")
                  nc.vector.scalar_tensor_tensor(
                      out=elrS[:], in0=elinS[:], scalar=NEG_SLOPE, in1=elinS[:],
                      op0=ALU.mult, op1=ALU.max)
                  eexpSx = wk.tile([P, ACOLS], bf16, tag="eexpSx", bufs=2,
                                   name="eexpSx")
                  nc.scalar.activation(
                      eexpSx[:].rearrange("p (h q) -> p h q", h=HEADS),
                      elrS[:, :, None].to_broadcast([P, HEADS, HID + 1]),
                      AFT.Exp)
                  rhsaS = wk.tile([P, ACOLS], bf16, tag="rhsaS", bufs=2,
                                  name="rhsaS")
                  nc.vector.tensor_tensor(
                      out=rhsaS[:], in0=rself[:, 0:ACOLS], in1=eexpSx[:],
                      op=ALU.mult)
                  nc.tensor.matmul(out=psB[:], lhsT=IDENT_sb[:], rhs=rhsaS[:],
                                   start=True, stop=False)
                  # logit chain: asrc (gathered) + adp (precomputed)
                  elin = wk.tile([P, S * HEADS], f32, tag="elin", bufs=2,
                                 name="elin")
                  nc.vector.tensor_tensor(
                      out=elin[:].rearrange("p (t h) -> p t h", h=HEADS),
                      in0=g[:, :, ACOLS:ACOLS + HEADS],
                      in1=adp_sb[:, c * S * HEADS:(c + 1) * S * HEADS
                                 ].rearrange("p (t h) -> p t h", h=HEADS),
                      op=ALU.add)
                  elr = wk.tile([P, S * HEADS], f32, tag="elr", bufs=2,
                                name="elr")
                  nc.vector.scalar_tensor_tensor(
                      out=elr[:], in0=elin[:], scalar=NEG_SLOPE, in1=elin[:],
                      op0=ALU.mult, op1=ALU.max)
                  # broadcast-exp on the scalar engine: expanded bf16 alpha so
                  # the row scaling below runs in the DVE 4x perf mode.
                  # Emitted in pieces so Act/DVE/PE pipeline within the chunk.
                  eexpx = wk.tile([P, S, ACOLS], bf16, tag="eexpx", bufs=2,
                                  name="eexpx")
                  rhsa = wk.tile([P, S, ACOLS], bf16, tag="rhsa", bufs=2,
                                 name="rhsa")
                  QP = 5                      # subtiles per piece
                  for q0 in range(0, S, QP):
                      q1 = min(q0 + QP, S)
                      nc.scalar.activation(
                          eexpx[:, q0:q1, :].rearrange(
                              "p t (h q) -> p t h q", h=HEADS),
                          elr[:, q0 * HEADS:q1 * HEADS].rearrange(
                              "p (t h) -> p t h", h=HEADS)[
                              :, :, :, None].to_broadcast(
                                  [P, q1 - q0, HEADS, HID + 1]),
                          AFT.Exp)
                      nc.vector.tensor_tensor(
                          out=rhsa[:, q0:q1, :], in0=g[:, q0:q1, 0:ACOLS],
                          in1=eexpx[:, q0:q1, :], op=ALU.mult)
                  for t in range(S):
                      col = c * S + t
                      s0 = wk.tile([P, P], bf16, tag="s0", bufs=4, name="s0")
                      nc.vector.tensor_scalar(
                          out=s0[:], in0=IOTA_sb[:],
                          scalar1=EDSTL_sb[:, col:col + 1], scalar2=None,
                          op0=ALU.is_equal)
                      nc.tensor.matmul(out=psB[:], lhsT=s0[:],
                                       rhs=rhsa[:, t, :],
                                       start=False, stop=(t == S - 1))
                  # epilogue: normalize + bias + relu -> h1 chunk
                  den = wk.tile([P, HEADS], f32, tag="den", name="den")
                  nc.vector.tensor_scalar(
                      out=den[:], in0=psB[:, HID::HID + 1].to_broadcast([P, HEADS]),
                      scalar1=EPS, scalar2=None, op0=ALU.add)
                  rec = wk.tile([P, HEADS], f32, tag="rec", name="rec")
                  nc.vector.reciprocal(rec[:], den[:])
                  h1n = wk.tile([P, C1], bf16, tag="h1n", name="h1n")
                  nc.vector.tensor_tensor(
                      out=h1n[:].rearrange("p (h q) -> p h q", h=HEADS),
                      in0=psB[:].rearrange("p (h q) -> p h q", h=HEADS)[:, :, 0:HID],
                      in1=rec[:, :, None].to_broadcast([P, HEADS, HID]),
                      op=ALU.mult)
                  nc.vector.tensor_tensor(out=h1n[:], in0=h1n[:], in1=B1R_sb[:],
                                          op=ALU.add)
                  nc.vector.tensor_scalar(
                      out=h1sh[:, c * C1:(c + 1) * C1], in0=h1n[:],
                      scalar1=0.0, scalar2=None, op0=ALU.max)

                  # ========== stage B' (interleaved): node table 2 ==========
                  if "B2" in stages:
                      i = c
                      h1t = h1sh[:, i * C1:(i + 1) * C1]
                      q0 = ps.tile([P, P], bf16, tag="tr", bufs=4, name="q0")
                      q1 = ps.tile([P, P], bf16, tag="tr", bufs=4, name="q1")
                      nc.tensor.transpose(out=q0[:], in_=h1t[:, 0:P], identity=IDENT_sb[:])
                      nc.tensor.transpose(out=q1[:], in_=h1t[:, P:C1], identity=IDENT_sb[:])
                      ht0 = wk.tile([P, P], bf16, tag="hc0", name="ht0")
                      ht1 = wk.tile([P, P], bf16, tag="hc1", name="ht1")
                      nc.vector.tensor_copy(ht0[:], q0[:])
                      nc.vector.tensor_copy(ht1[:], q1[:])
                      h2p = ps.tile([HID, P], f32, tag="tr", bufs=4, name="h2p")
                      nc.tensor.matmul(out=h2p[:], lhsT=W2h_sb[:, 0:HID], rhs=ht0[:],
                                       start=True, stop=False)
                      nc.tensor.matmul(out=h2p[:], lhsT=W2h_sb[:, HID:2 * HID],
                                       rhs=ht1[:], start=False, stop=True)
                      h2t = wk.tile([HID, P], bf16, tag="h2t", name="h2t")
                      nc.vector.tensor_copy(h2t[:], h2p[:])
                      aa2 = ps.tile([2, P], f32, tag="sm", bufs=2, name="aa2")
                      nc.tensor.matmul(out=aa2[:], lhsT=A2_sb[:], rhs=h2t[:],
                                       start=True, stop=True)
                      aa2_sb = wk.tile([2, P], bf16, tag="aa2_sb", name="aa2_sb")
                      nc.vector.tensor_copy(aa2_sb[:], aa2[:])
                      r2h = ps.tile([P, HID], bf16, tag="tr", bufs=4, name="r2h")
                      nc.tensor.transpose(out=r2h[:], in_=h2t[:], identity=IDENT_sb[:HID, :HID])
                      r2a = ps.tile([P, 2], bf16, tag="sm", bufs=2, name="r2a")
                      nc.tensor.transpose(out=r2a[:], in_=aa2_sb[:], identity=IDENT_sb[:2, :2])
                      row2 = wk.tile([P, ROW2], bf16, tag="row2", name="row2")
                      nc.vector.memset(row2[:, HID + 2:ROW2], 0.0)
                      nc.vector.tensor_copy(row2[:, 0:HID], r2h[:])
                      nc.vector.memset(row2[:, HID:HID + 1], 1.0)
                      nc.vector.tensor_copy(row2[:, HID + 1:HID + 2], r2a[:, 0:1])
                      nc.vector.tensor_copy(adst2_sh[:, i:i + 1], r2a[:, 1:2])
                      nc.sync.dma_start(tb2s[i * P:(i + 1) * P, :], row2[:])

              if "B2" in stages:
                  nc.gpsimd.collective_compute(
                      "AllGather", ALU.bypass, replica_groups=rg,
                      ins=[tb2s[:]], outs=[tb2[:]])

              # ========== prep-2 (overlaps AG2): adp2 for layer 2 ==========
              tc.tile_set_cur_wait(_PH + 3)
              for c in range(NCHUNK if "C" in stages else 0):
                  edt2 = gp.tile([P, S * P], mybir.dt.int8, tag="edt", name="edt2")
                  nc.sync.dma_start(edt2[:],
                                    t_EDSTLT.ap()[:, c * S * P:(c + 1) * S * P])
                  adp2_ps = ps.tile([P, S], f32, tag="sm", bufs=2,
                                    name="adp2_ps")
                  for t in range(S):
                      s0t2 = wk.tile([P, P], bf16, tag="s0t", bufs=4, name="s0t2")
                      nc.vector.tensor_scalar(
                          out=s0t2[:], in0=edt2[:, t * P:(t + 1) * P],
                          scalar1=IOTAT_sb[:, 0:1], scalar2=None,
                          op0=ALU.is_equal)
                      nc.tensor.matmul(out=adp2_ps[:, t:t + 1], lhsT=s0t2[:],
                                       rhs=adst2_sh[:, c:c + 1],
                                       start=True, stop=True)
                  nc.scalar.copy(adp2_sb[:, c * S:(c + 1) * S], adp2_ps[:])

              if "C" not in stages and "B" in stages:
                  keep = wk.tile([P, OUT_CH], f32, tag="yout", name="keep")
                  nc.vector.tensor_copy(keep[:], h1sh[:, 0:OUT_CH])
                  nc.sync.dma_start(t_Y.ap()[0:P, :], keep[:])

              # =================== stage C: layer-2 aggregation + final ====
              tc.tile_set_cur_wait(_PH + 4)
              for c in range(NCHUNK if "C" in stages else 0):
                  g2 = gp.tile([P, S, ROW2], bf16, tag="g2", bufs=3, name="g2")
                  nc.gpsimd.dma_gather(
                      out_ap=g2[:, 0:S_LO, :], in_ap=tb2[0:HALF, :],
                      idxs_ap=IDXM_sb[:, c * S * 8:c * S * 8 + S_LO * 8],
                      num_idxs=S_LO * P, num_idxs_reg=S_LO * P,
                      elem_size=ROW2, single_packet=False)
                  nc.gpsimd.dma_gather(
                      out_ap=g2[:, S_LO:S, :], in_ap=tb2[HALF:2 * HALF, :],
                      idxs_ap=IDXM_sb[:, c * S * 8 + S_LO * 8:(c + 1) * S * 8],
                      num_idxs=S_HI * P, num_idxs_reg=S_HI * P,
                      elem_size=ROW2, single_packet=False)
                  psC = ps.tile([P, HID + 1], f32, tag="agg", bufs=2, name="psC")
                  # self-loop contribution from local rows (no gather)
                  rself2 = gp.tile([P, ROW2], bf16, tag="rself2", name="rself2")
                  nc.sync.dma_start(rself2[:], tb2s[c * P:(c + 1) * P, :])
                  elinS2 = wk.tile([P, 1], f32, tag="elinS", bufs=2,
                                   name="elinS2")
                  nc.vector.tensor_tensor(
                      out=elinS2[:], in0=rself2[:, HID + 1:HID + 2],
                      in1=adst2_sh[:, c:c + 1], op=ALU.add)
                  elrS2 = wk.tile([P, 1], f32, tag="elrS", bufs=2, name="elrS2")
                  nc.vector.scalar_tensor_tensor(
                      out=elrS2[:], in0=elinS2[:], scalar=NEG_SLOPE,
                      in1=elinS2[:], op0=ALU.mult, op1=ALU.max)
                  eexpS2 = wk.tile([P, 1], f32, tag="eexpS2", bufs=2,
                                   name="eexpS2")
                  nc.scalar.activation(eexpS2[:], elrS2[:], AFT.Exp)
                  s0S = wk.tile([P, P], bf16, tag="s0w", bufs=6, name="s0S")
                  nc.vector.tensor_scalar(
                      out=s0S[:], in0=IDENT_sb[:], scalar1=eexpS2[:, 0:1],
                      scalar2=None, op0=ALU.mult)
                  nc.tensor.matmul(out=psC[:], lhsT=s0S[:],
                                   rhs=rself2[:, 0:HID + 1],
                                   start=True, stop=False)
                  # batched logit chain (adp2 precomputed during AG2)
                  elin2 = wk.tile([P, S], f32, tag="elin", bufs=2, name="elin2")
                  nc.vector.tensor_tensor(
                      out=elin2[:, :, None], in0=g2[:, :, HID + 1:HID + 2],
                      in1=adp2_sb[:, c * S:(c + 1) * S, None], op=ALU.add)
                  elr2 = wk.tile([P, S], f32, tag="elr", bufs=2, name="elr2")
                  nc.vector.scalar_tensor_tensor(
                      out=elr2[:], in0=elin2[:], scalar=NEG_SLOPE, in1=elin2[:],
                      op0=ALU.mult, op1=ALU.max)
                  eexp2 = wk.tile([P, S], f32, tag="eexp", bufs=2, name="eexp2")
                  nc.scalar.activation(eexp2[:], elr2[:], AFT.Exp)
                  # pass 2: fused scaled one-hot (is_equal * eexp2) + matmul
                  for t in range(S):
                      col = c * S + t
                      s0w = wk.tile([P, P], bf16, tag="s0w", bufs=6, name="s0w")
                      nc.vector.tensor_scalar(
                          out=s0w[:], in0=IOTA_sb[:],
                          scalar1=EDSTL_sb[:, col:col + 1],
                          scalar2=eexp2[:, t:t + 1],
                          op0=ALU.is_equal, op1=ALU.mult)
                      nc.tensor.matmul(out=psC[:], lhsT=s0w[:],
                                       rhs=g2[:, t, 0:HID + 1],
                                       start=False, stop=(t == S - 1))
                  den2 = wk.tile([P, 1], f32, tag="den", name="den2")
                  nc.vector.tensor_scalar(out=den2[:], in0=psC[:, HID:HID + 1],
                                          scalar1=EPS, scalar2=None, op0=ALU.add)
                  rec2 = wk.tile([P, 1], f32, tag="rec", name="rec2")
                  nc.vector.reciprocal(rec2[:], den2[:])
                  h2n = wk.tile([P, HID], bf16, tag="h1n", name="h2n")
                  nc.vector.tensor_scalar(out=h2n[:], in0=psC[:, 0:HID],
                                          scalar1=rec2[:, 0:1], scalar2=None,
                                          op0=ALU.mult)
                  nc.vector.tensor_tensor(out=h2n[:], in0=h2n[:], in1=B2R_sb[:],
                                          op=ALU.add)
                  th2 = ps.tile([HID, P], bf16, tag="tr", bufs=4, name="th2")
                  nc.tensor.transpose(out=th2[:], in_=h2n[:],
                                      identity=IDENT_sb[:])
                  th2s = wk.tile([HID, P], bf16, tag="h2t", name="th2s")
                  nc.scalar.copy(th2s[:], th2[:])
                  yo = ps.tile([P, OUT_CH], f32, tag="tr", bufs=4, name="yo")
                  nc.tensor.matmul(out=yo[:], lhsT=th2s[:], rhs=WL_sb[:],
                                   start=True, stop=True)
                  yout = wk.tile([P, OUT_CH], f32, tag="yout", name="yout")
                  nc.vector.tensor_tensor(out=yout[:], in0=yo[:], in1=BLR_sb[:],
                                          op=ALU.add)
                  nc.sync.dma_start(t_Y.ap()[c * P:(c + 1) * P, :], yout[:])

    nc.compile()
    return nc


# ---------------------------------------------------------------- runner
class Runner:
    """Cached PJRT runner: jits once per compiled nc, keeps inputs
    device-resident. Much faster than run_bass_kernel_spmd for repeat calls
    and lets wall-clock approximate device exec time."""

    def __init__(self, nc, n_cores):
        import jax
        from jax.sharding import Mesh, PartitionSpec
        from jax.experimental.shard_map import shard_map
        import concourse.mybir as mybir
        from concourse import bass2jax
        self._jax = jax
        bass2jax.install_neuronx_cc_hook()
        partition_name = (nc.partition_id_tensor.name
                          if nc.partition_id_tensor else None)
        dbg_name = nc.dbg_addr.name if nc.dbg_addr else None
        in_names, out_names, out_avals, zero_outs = [], [], [], []
        for alloc in nc.m.functions[0].allocations:
            if not isinstance(alloc, mybir.MemoryLocationSet):
                continue
            name = alloc.memorylocations[0].name
            if alloc.kind == "ExternalInput":
                if name not in (partition_name, dbg_name):
                    in_names.append(name)
            elif alloc.kind == "ExternalOutput":
                out_names.append(name)
                shape = tuple(alloc.tensor_shape)
                dtype = mybir.dt.np(alloc.dtype)
                out_avals.append(jax.core.ShapedArray(shape, dtype))
                zero_outs.append(np.zeros(shape, dtype))
        self.n_cores = n_cores
        self.in_names = in_names
        self.out_names = out_names
        self.out_avals = out_avals
        self.zero_outs = zero_outs
        n_params = len(in_names)
        n_outs = len(out_names)
        all_in = list(in_names) + list(out_names)
        if dbg_name is not None:
            all_in.append(dbg_name)
        if partition_name is not None:
            all_in.append(partition_name)

        def _body(*args):
            operands = list(args)
            if dbg_name is not None:
                operands.append(jax.numpy.zeros((1, 2), jax.numpy.uint32))
            if partition_name is not None:
                operands.append(bass2jax.partition_id_tensor())
            return tuple(bass2jax._bass_exec_p.bind(
                *operands, out_avals=tuple(out_avals), in_names=tuple(all_in),
                out_names=tuple(out_names), lowering_input_output_aliases=(),
                sim_require_finite=True, sim_require_nnan=True, nc=nc))

        devices = jax.devices()[:n_cores]
        assert len(devices) == n_cores
        if n_cores == 1:
            self.fn = jax.jit(_body, keep_unused=True)
        else:
            mesh = Mesh(np.asarray(devices), ("core",))
            in_specs = (PartitionSpec("core"),) * (n_params + n_outs)
            out_specs = (PartitionSpec("core"),) * n_outs
            self.fn = jax.jit(
                shard_map(_body, mesh=mesh, in_specs=in_specs,
                          out_specs=out_specs, check_rep=False),
                keep_unused=True)
        self._dev_in = None

    def set_inputs(self, in_maps):
        jax = self._jax
        per_core = [[np.asarray(m[n]) for n in self.in_names] for m in in_maps]
        n_params = len(self.in_names)
        if self.n_cores == 1:
            arrs = [per_core[0][i] for i in range(n_params)]
            zer = list(self.zero_outs)
        else:
            arrs = [np.concatenate([per_core[c][i] for c in range(self.n_cores)],
                                   axis=0) for i in range(n_params)]
            zer = [np.zeros((self.n_cores * z.shape[0], *z.shape[1:]), z.dtype)
                   for z in self.zero_outs]
        self._dev_in = [jax.device_put(a) for a in arrs + zer]

    def run(self):
        outs = self.fn(*self._dev_in)
        self._jax.block_until_ready(outs)
        return outs

    def results(self, outs):
        res = []
        for c in range(self.n_cores):
            d = {}
            for i, name in enumerate(self.out_names):
                a = np.asarray(outs[i])
                if self.n_cores > 1:
                    a = a.reshape(self.n_cores, *self.out_avals[i].shape)[c]
                d[name] = a
            res.append(d)
        return res



_CACHE = {}


def _get_built(cfg_key, cfg):
    if cfg_key not in _CACHE:
        _CACHE[cfg_key] = build(cfg)
    return _CACHE[cfg_key]


def _get_runner(cfg, reps=1):
    key = ("runner", cfg["S"], cfg["S_LO"], reps)
    if key not in _CACHE:
        nc = build(cfg, reps=reps)
        _CACHE[key] = Runner(nc, cfg["NC"])
    return _CACHE[key]


def kernel(**inputs) -> np.ndarray:
    cfg = _cfg_full()
    in_maps, cfg = host_prep(inputs, cfg)
    r = _get_runner(cfg)
    r.set_inputs(in_maps)
    res = r.results(r.run())
    NSH = cfg["NSH"]
    newpos = cfg["NEWPOS"]
    y = np.empty((cfg["NC"] * NSH, OUT_CH), np.float32)
    for c in range(cfg["NC"]):
        nodes_c = np.arange(c * NSH, (c + 1) * NSH)
        y[nodes_c] = res[c]["Y"][newpos[nodes_c]]
    return np.ascontiguousarray(y, dtype=np.float32)


if __name__ == "__main__":
    import reference as R
    inp = R.setup_inputs()
    out = kernel(**{k: np.asarray(v) for k, v in inp.items()})
    exp = np.asarray(R.reference(**inp))
    err = np.abs(out - exp).max() / (np.abs(exp).max() + 1e-12)
    print("rel err:", err)

